# revision 3
# baseline (speedup 1.0000x reference)
"""CAM (channel attention) module kernel for Trainium2 (Bass/Tile).

Reference computation (per batch b):
    energy  = x_b @ x_b.T                      # [C, C], contraction over N
    att     = softmax(rowmax(energy) - energy) # row-wise over last axis
    out     = att @ x_b                        # [C, N]
    y_b     = gamma * out + x_b
Identity used: softmax(rowmax(E) - E)[i,j] = exp(mn[i] - E[i,j]) / Z[i]
with mn[i] = min_j E[i,j] (shift invariance of softmax; exact).

Sharding: data-parallel over B across 8 NeuronCores (B=32 -> 4 per core),
gamma replicated, full CxC attention per core.

fp8 design (v2): both matmuls run fp8e4 with perf_mode=DoubleRow (two
K=128 subtiles per matmul, 2 MACs/cell/cycle):
    - xt (x transposed) is built as [P, 2, C] fp8 k-chunk PAIRS: PE
      transposes write PSUM (fp8 from X8 for kc>=bf_cut, f32 from X
      below it -- the f32 path has no cast dependency so next-batch
      prefetch never waits on DVE), evac converts to fp8 pair halves.
    - mm1: E += xt_pair[:,e,:].T @ xt_pair[:,e,:] summed over e, upper
      triangle only; mirrored via PE transpose.
    - softmax on f32 E in PSUM: DVE row-min, ScalarE exp (bf16 tS, Z
      fused), rg = gamma/Z.
    - tT: PE-transpose tS -> bf16 PSUM, ScalarE evac converts to fp8.
    - mm2: per 512-col block, 2 DoubleRow matmuls over jc-pairs with
      moving operand X8 [P, 2, 512]; evac: ScalarE scales by rg,
      residual add of f32 X on res_engine; DMA out.
The residual path stays f32 end to end: rg*psum + x with rg = gamma/Z,
so gamma = 0 reproduces x exactly (all intermediate values finite).

Cross-batch software pipeline (PE program order per steady-state batch):
    [trans(b) kc>=PRE interleaved with mm1(b) pairs] -> mirror(b) ->
    [trans(b+1) kc<PRE : fills the softmax(b) latency] -> tT(b) -> mm2(b)
"""

import contextlib

import numpy as np

P = 128

_CACHE = {}


DEFAULT_OPTS = dict(
    pre=8,         # k-chunks of next batch's transposes emitted early (even)
    xt_bufs=6,     # xT pair SBUF tiles
    o_bufs=8,      # output staging tiles
    cast_engine="vector",   # engine for f32->fp8 natural-layout cast
    evac_engine="scalar",   # engine for ps_x -> xt evacuation
    evac_alt=True,          # alternate xt evac between scalar and vector
    res_engine="gpsimd",    # engine for the f32 residual add
    bf_cut=10,     # kc >= bf_cut transpose X8 (fp8); below: X (f32)
    ts_bf16=True,  # tS (exp output) in bf16
    timing_io=False,
)


def _build(Bs, C, N, use_f32r=False, reps=1, **opts):
    import concourse.bass as bass  # noqa: F401
    import concourse.tile as tile
    import concourse.mybir as mybir
    from concourse import bacc
    from concourse.masks import make_identity

    o = dict(DEFAULT_OPTS)
    o.update(opts)

    F32 = mybir.dt.float32
    BF16 = mybir.dt.bfloat16
    FP8 = mybir.dt.float8e4
    AF = mybir.ActivationFunctionType
    ALU = mybir.AluOpType
    AX = mybir.AxisListType
    DR = mybir.MatmulPerfMode.DoubleRow

    assert C == 4 * P and N % 512 == 0
    CO = C // P          # i/j chunks of 128
    KC = N // P          # n chunks of 128 (contraction for energy)
    KP = KC // 2         # k-chunk pairs for DoubleRow
    NF = N // 512        # n chunks of 512 (DMA / matmul-2 free dim)
    PRE = o["pre"]
    assert KC % 2 == 0 and PRE % 2 == 0
    TSDT = BF16 if o["ts_bf16"] else F32

    nc = bacc.Bacc(None, target_bir_lowering=False, debug=False)
    x_in = nc.dram_tensor("x", [Bs, C, N], F32, kind="ExternalInput")
    g_in = nc.dram_tensor("gamma", [1], F32, kind="ExternalInput")
    y_out = nc.dram_tensor("y", [Bs, C, N], F32, kind="ExternalOutput")

    with tile.TileContext(nc) as tc:
        with (
            tc.tile_pool(name="consts", bufs=1) as consts,
            tc.tile_pool(name="xpool", bufs=2) as xpool,
            tc.tile_pool(name="x8pool", bufs=1) as x8pool,
            tc.tile_pool(name="xtp", bufs=o["xt_bufs"]) as xtp,
            tc.tile_pool(name="tpool", bufs=1) as tpool,
            tc.tile_pool(name="ttpool", bufs=1) as ttpool,
            tc.tile_pool(name="opool", bufs=o["o_bufs"]) as opool,
            tc.tile_pool(name="stats", bufs=1) as stats,
            tc.tile_pool(name="pe", bufs=1, space="PSUM") as psum_e,
            tc.tile_pool(name="pxt", bufs=2, space="PSUM") as psum_xt,
            tc.tile_pool(name="pacc", bufs=2, space="PSUM") as psum_acc,
        ):
            ident = consts.tile([P, P], F32)
            make_identity(nc, ident)
            ident16 = consts.tile([P, P], BF16)
            nc.vector.tensor_copy(ident16[:, :], ident[:, :])
            ident8 = consts.tile([P, P], FP8)
            nc.vector.tensor_copy(ident8[:, :], ident[:, :])
            g_sb = consts.tile([1, 1], F32)
            nc.sync.dma_start(g_sb[:, :], g_in[:].rearrange("(a b) -> a b", a=1))
            g_col = consts.tile([P, 1], F32)
            nc.gpsimd.partition_broadcast(g_col[:, :], g_sb[:1, :1])

            def eng_copy(engine, out, in_):
                if engine == "vector":
                    nc.vector.tensor_copy(out, in_)
                elif engine == "scalar":
                    nc.scalar.copy(out, in_)
                else:
                    nc.gpsimd.tensor_copy(out, in_)

            # per-batch live tiles
            Xs, X8s, xts = {}, {}, {}

            def emit_dma(b):
                """DMA x_b in 512-col chunks."""
                x_b = x_in[b].rearrange("(co p) n -> p co n", p=P)
                X = xpool.tile([P, CO, N], F32, tag="X")
                Xs[b] = X
                # first 128 columns land alone so transposes start early
                nc.sync.dma_start(X[:, :, 0:P], x_b[:, :, 0:P])
                nc.sync.dma_start(X[:, :, P:512], x_b[:, :, P:512])
                for nf in range(1, NF):
                    s = slice(nf * 512, (nf + 1) * 512)
                    nc.sync.dma_start(X[:, :, s], x_b[:, :, s])

            def emit_cast(b):
                """f32->fp8 natural-layout copy: matmul-2's moving operand
                and the fp8-transpose source. Emitted at the start of
                batch b's own cycle (single X8 buffer free right then).
                Chunk order starts at bf_cut's chunk so the first fp8
                transposes of phase A unblock earliest."""
                X = Xs[b]
                X8 = x8pool.tile([P, CO, N], FP8, tag="X8")
                X8s[b] = X8
                first_chunk = o["bf_cut"] // 4
                order = [(first_chunk + i) % NF for i in range(NF)]
                for nf in order:
                    s = slice(nf * 512, (nf + 1) * 512)
                    for co in range(CO):
                        eng_copy(o["cast_engine"], X8[:, co, s], X[:, co, s])

            def emit_trans(b, kc, evac=None):
                """Transpose the kc-th 128-col slab of x_b into half of an
                fp8 xt pair tile. kc >= bf_cut reads fp8 X8; below reads
                f32 X (no cast dependency -- used by the cross-batch
                filler and the first A-phase groups)."""
                src8 = kc >= o["bf_cut"]
                ks = slice(kc * P, (kc + 1) * P)
                t = kc // 2
                if (b, t) not in xts:
                    xts[(b, t)] = xtp.tile([P, 2, C], FP8, tag="xt",
                                           name="xt_pair")
                xt_pair = xts[(b, t)]
                src = X8s[b] if src8 else Xs[b]
                idt = ident8 if src8 else ident
                # fp8 transposes must write with element step 2 (HW rule),
                # so the PSUM tile is viewed as [P, 2, C] with the value in
                # slot 0 of each 2-byte granule; the evac read de-interleaves.
                # Tiles padded to a full 2KB PSUM bank so the psx ring
                # buffers never share a bank (PE-W vs evac-R collide at
                # bank granularity)
                if src8:
                    ps_x = psum_xt.tile(
                        [P, 2 * C], FP8, tag="psx", name="ps_x",
                        padded_shape=[P, 2048],
                    )
                    v = ps_x.rearrange("p (c two) -> p two c", two=2)
                    for co in range(CO):
                        nc.tensor.transpose(
                            v[:, 0, co * P:(co + 1) * P], src[:, co, ks], idt
                        )
                    ps_rd = v[:, 0, :]
                else:
                    ps_x = psum_xt.tile([P, C], F32, tag="psx", name="ps_x")
                    for co in range(CO):
                        nc.tensor.transpose(
                            ps_x[:, co * P:(co + 1) * P], src[:, co, ks], idt
                        )
                    ps_rd = ps_x[:, :]
                eng = evac or o["evac_engine"]
                if o["evac_alt"] and evac is None:
                    eng = "scalar" if kc % 2 == 0 else "vector"
                eng_copy(eng, xt_pair[:, kc % 2, :], ps_rd)

            def emit_mm1(b, t, E):
                xt_pair = xts.pop((b, t))
                for ic in range(CO):
                    nc.tensor.matmul(
                        E[:, ic, ic * P:],
                        xt_pair[:, :, ic * P:(ic + 1) * P],
                        xt_pair[:, :, ic * P:],
                        start=(t == 0),
                        stop=(t == KP - 1),
                        perf_mode=DR,
                    )

            def emit_mirror(b, E):
                for jc in range(1, CO):
                    for ic in range(jc):
                        stg = xtp.tile([P, P], F32, tag="mirror_stage",
                                       bufs=1)
                        nc.scalar.copy(
                            stg[:, :], E[:, ic, jc * P:(jc + 1) * P]
                        )
                        nc.tensor.matmul(
                            E[:, jc, ic * P:(ic + 1) * P],
                            stg[:, :],
                            ident,
                            is_transpose=True,
                            skip_group_check=True,
                        )

            def emit_softmax(b, E):
                mn = stats.tile([P, CO], F32, tag="mn")
                zs = stats.tile([P, CO], F32, tag="zs")
                rg = stats.tile([P, CO], F32, tag="rg")
                tS = tpool.tile([P, CO, C], TSDT, tag="t")
                for ic in range(CO):
                    nc.vector.tensor_reduce(
                        mn[:, ic:ic + 1], E[:, ic, :], AX.X, ALU.min
                    )
                for ic in range(CO):
                    nc.scalar.activation(
                        tS[:, ic, :], E[:, ic, :], AF.Exp,
                        bias=mn[:, ic:ic + 1], scale=-1.0,
                        accum_out=zs[:, ic:ic + 1],
                    )
                nc.vector.reciprocal(rg[:, :], zs[:, :])
                nc.vector.tensor_scalar_mul(rg[:, :], rg[:, :], g_col[:, :1])
                return tS, rg

            def emit_tT(b, tS):
                tT = ttpool.tile([P, CO, C], FP8, tag="tT")
                idt = ident16 if o["ts_bf16"] else ident
                for jc in range(CO):
                    ps_t = psum_acc.tile(
                        [P, C], TSDT, tag="acc", name="ps_t",
                        padded_shape=[P, 1024] if o["ts_bf16"] else None,
                    )
                    for ic in range(CO):
                        nc.tensor.transpose(
                            ps_t[:, ic * P:(ic + 1) * P],
                            tS[:, ic, jc * P:(jc + 1) * P],
                            idt,
                        )
                    nc.scalar.copy(tT[:, jc, :], ps_t[:, :])
                return tT

            def emit_mm2(b, tT, rg):
                X, X8 = Xs[b], X8s[b]
                y_b = y_out[b].rearrange("(co p) n -> p co n", p=P)
                for ic in range(CO):
                    for nf in range(NF):
                        ns = slice(nf * 512, (nf + 1) * 512)
                        g = ic * NF + nf
                        if g % 3 == 2:
                            # E's PSUM region is dead during mm2 (softmax
                            # already read it); borrowing it as a third
                            # rotation slot hides the evac latency behind
                            # two full matmul groups
                            ps2 = psum_e.tile([P, C], F32, tag="E",
                                              name="ps2e")
                        else:
                            ps2 = psum_acc.tile([P, C], F32, tag="acc")
                        for t in range(2):
                            nc.tensor.matmul(
                                ps2[:, :512],
                                tT[:, 2 * t:2 * t + 2, ic * P:(ic + 1) * P],
                                X8[:, 2 * t:2 * t + 2, ns],
                                start=(t == 0),
                                stop=(t == 1),
                                perf_mode=DR,
                            )
                        ot = opool.tile([P, 512], F32, tag="o")
                        nc.scalar.activation(
                            ot[:, :], ps2[:, :512], AF.Copy,
                            bias=0.0, scale=rg[:, ic:ic + 1],
                        )
                        if o["res_engine"] == "gpsimd":
                            nc.gpsimd.tensor_add(
                                ot[:, :], ot[:, :], X[:, ic, ns]
                            )
                        else:
                            nc.vector.tensor_add(
                                ot[:, :], ot[:, :], X[:, ic, ns]
                            )
                        nc.sync.dma_start(y_b[:, ic, ns], ot[:, :])
                del Xs[b], X8s[b]

            loop_ctx = (
                tc.For_i(0, reps, 1) if reps > 1 else contextlib.nullcontext()
            )
            with loop_ctx:
                emit_dma(0)
                for b in range(Bs):
                    first = (b == 0)
                    emit_cast(b)
                    E = psum_e.tile([P, CO, C], F32, tag="E")
                    if first:
                        # no prefetched transposes: run 2 ahead of mm1
                        emit_trans(b, 0)
                        emit_trans(b, 1)
                        for kc in range(KC):
                            if kc + 2 < KC:
                                emit_trans(b, kc + 2)
                            if kc == 16 and b + 1 < Bs:
                                emit_dma(b + 1)
                            if kc % 2 == 1:
                                emit_mm1(b, kc // 2, E)
                    else:
                        # kc < PRE were transposed during softmax(b-1)
                        for kc in range(KC):
                            if kc == 0 and b + 1 < Bs:
                                emit_dma(b + 1)
                            if kc + PRE < KC:
                                emit_trans(b, kc + PRE)
                            if kc % 2 == 1:
                                emit_mm1(b, kc // 2, E)
                    emit_mirror(b, E)
                    tS, rg = emit_softmax(b, E)
                    if b + 1 < Bs:
                        for kc in range(PRE):
                            emit_trans(b + 1, kc)
                    tT = emit_tT(b, tS)
                    emit_mm2(b, tT, rg)

    nc.compile()
    return nc


def get_nc(Bs=4, C=512, N=4096, use_f32r=False, reps=1, **opts):
    key = (Bs, C, N, use_f32r, reps, tuple(sorted(opts.items())))
    if key not in _CACHE:
        _CACHE[key] = _build(Bs, C, N, use_f32r, reps, **opts)
    return _CACHE[key]


def kernel(x, gamma):
    """Full inputs in, full output out. x [32, 512, 4096] f32, gamma [1] f32."""
    from concourse.bass_utils import run_bass_kernel_spmd

    x = np.ascontiguousarray(np.asarray(x, dtype=np.float32))
    gamma = np.ascontiguousarray(np.asarray(gamma, dtype=np.float32))
    B, C, N = x.shape
    n_cores = 8
    assert B % n_cores == 0
    Bs = B // n_cores

    nc = get_nc(Bs, C, N)
    in_maps = [
        {"x": x[i * Bs:(i + 1) * Bs], "gamma": gamma} for i in range(n_cores)
    ]
    res = run_bass_kernel_spmd(nc, in_maps, core_ids=list(range(n_cores)))
    return np.concatenate([r["y"] for r in res.results], axis=0)


# revision 12
# speedup vs baseline: 1.1206x; 1.1206x over previous
"""CAM (channel attention) module kernel for Trainium2 (Bass/Tile).

Reference computation (per batch b):
    energy  = x_b @ x_b.T                      # [C, C], contraction over N
    att     = softmax(rowmax(energy) - energy) # row-wise over last axis
    out     = att @ x_b                        # [C, N]
    y_b     = gamma * out + x_b
Identity used: softmax(rowmax(E) - E)[i,j] = exp(mn[i] - E[i,j]) / Z[i]
with mn[i] = min_j E[i,j] (shift invariance of softmax; exact).

Sharding: data-parallel over B across 8 NeuronCores (B=32 -> 4 per core),
gamma replicated, full CxC attention per core.

fp8 design (v2): both matmuls run fp8e4 with perf_mode=DoubleRow (two
K=128 subtiles per matmul, 2 MACs/cell/cycle):
    - xt (x transposed) is built as [P, 2, C] fp8 k-chunk PAIRS: PE
      transposes write PSUM (fp8 from X8 for kc>=bf_cut, f32 from X
      below it -- the f32 path has no cast dependency so next-batch
      prefetch never waits on DVE), evac converts to fp8 pair halves.
    - mm1: E += xt_pair[:,e,:].T @ xt_pair[:,e,:] summed over e, upper
      triangle only; mirrored via PE transpose.
    - softmax on f32 E in PSUM: DVE row-min, ScalarE exp (bf16 tS, Z
      fused), rg = gamma/Z.
    - tT: PE-transpose tS -> bf16 PSUM, ScalarE evac converts to fp8.
    - mm2: per 512-col block, 2 DoubleRow matmuls over jc-pairs with
      moving operand X8 [P, 2, 512]; evac: ScalarE scales by rg,
      residual add of f32 X on res_engine; DMA out.
The residual path stays f32 end to end: rg*psum + x with rg = gamma/Z,
so gamma = 0 reproduces x exactly (all intermediate values finite).

Cross-batch software pipeline (PE program order per steady-state batch):
    [trans(b) kc>=PRE interleaved with mm1(b) pairs] -> mirror(b) ->
    [trans(b+1) kc<PRE : fills the softmax(b) latency] -> tT(b) -> mm2(b)
"""

import contextlib

import numpy as np

P = 128

_CACHE = {}


DEFAULT_OPTS = dict(
    pre=12,        # k-chunks of next batch's transposes emitted early (even)
    xt_bufs=9,     # xT pair SBUF tiles
    o_bufs=12,     # output staging tiles
    x8_bufs=2,     # X8 buffers (2 decouples cast(b) from mm2(b-1) reads)
    cast_engine="scalar",   # engine for f32->fp8 natural-layout cast
    evac_engine="scalar",   # engine for ps_x -> xt evacuation
    tt_evac="vector",       # engine for ps_t -> tT evacuation
    res_engine="vector",    # engine for the mm2 psum+residual add
    mm2_rot=4,     # mm2 PSUM bank rotation depth (2 acc + borrowed E banks)
    bf_cut=12,     # kc >= bf_cut transpose X8 (fp8); below: X (f32).
                   # Must be >= pre (prefetched transposes have no X8 yet).
    ts_bf16=True,  # tS (exp output) in bf16
    timing_io=False,
)


def _build(Bs, C, N, use_f32r=False, reps=1, **opts):
    import concourse.bass as bass  # noqa: F401
    import concourse.tile as tile
    import concourse.mybir as mybir
    from concourse import bacc
    from concourse.masks import make_identity

    o = dict(DEFAULT_OPTS)
    o.update(opts)

    F32 = mybir.dt.float32
    BF16 = mybir.dt.bfloat16
    FP8 = mybir.dt.float8e4
    AF = mybir.ActivationFunctionType
    ALU = mybir.AluOpType
    AX = mybir.AxisListType
    DR = mybir.MatmulPerfMode.DoubleRow

    assert C == 4 * P and N % 512 == 0
    CO = C // P          # i/j chunks of 128
    KC = N // P          # n chunks of 128 (contraction for energy)
    KP = KC // 2         # k-chunk pairs for DoubleRow
    NF = N // 512        # n chunks of 512 (DMA / matmul-2 free dim)
    PRE = o["pre"]
    assert KC % 2 == 0 and PRE % 2 == 0
    assert o["bf_cut"] >= PRE, "prefetched transposes need the f32 path"
    TSDT = BF16 if o["ts_bf16"] else F32

    nc = bacc.Bacc(None, target_bir_lowering=False, debug=False)
    x_in = nc.dram_tensor("x", [Bs, C, N], F32, kind="ExternalInput")
    g_in = nc.dram_tensor("gamma", [1], F32, kind="ExternalInput")
    y_out = nc.dram_tensor("y", [Bs, C, N], F32, kind="ExternalOutput")

    with tile.TileContext(nc) as tc:
        with (
            tc.tile_pool(name="consts", bufs=1) as consts,
            tc.tile_pool(name="xpool", bufs=2) as xpool,
            tc.tile_pool(name="x8pool", bufs=o["x8_bufs"]) as x8pool,
            tc.tile_pool(name="xtp", bufs=o["xt_bufs"]) as xtp,
            tc.tile_pool(name="tpool", bufs=1) as tpool,
            tc.tile_pool(name="ttpool", bufs=1) as ttpool,
            tc.tile_pool(name="opool", bufs=o["o_bufs"]) as opool,
            tc.tile_pool(name="stats", bufs=1) as stats,
            tc.tile_pool(name="pe", bufs=1, space="PSUM") as psum_e,
            tc.tile_pool(name="pxt", bufs=2, space="PSUM") as psum_xt,
            tc.tile_pool(name="pacc", bufs=2, space="PSUM") as psum_acc,
        ):
            ident = consts.tile([P, P], F32)
            make_identity(nc, ident)
            ident16 = consts.tile([P, P], BF16)
            nc.vector.tensor_copy(ident16[:, :], ident[:, :])
            ident8 = consts.tile([P, P], FP8)
            nc.vector.tensor_copy(ident8[:, :], ident[:, :])
            g_sb = consts.tile([1, 1], F32)
            nc.sync.dma_start(g_sb[:, :], g_in[:].rearrange("(a b) -> a b", a=1))
            g_col = consts.tile([P, 1], F32)
            nc.gpsimd.partition_broadcast(g_col[:, :], g_sb[:1, :1])

            def eng_copy(engine, out, in_):
                if engine == "vector":
                    nc.vector.tensor_copy(out, in_)
                elif engine == "scalar":
                    nc.scalar.copy(out, in_)
                else:
                    nc.gpsimd.tensor_copy(out, in_)

            # per-batch live tiles
            Xs, X8s, xts = {}, {}, {}

            def emit_dma(b):
                """DMA x_b in 512-col chunks."""
                x_b = x_in[b].rearrange("(co p) n -> p co n", p=P)
                X = xpool.tile([P, CO, N], F32, tag="X")
                Xs[b] = X
                # first 128 columns land alone so transposes start early
                nc.sync.dma_start(X[:, :, 0:P], x_b[:, :, 0:P])
                nc.sync.dma_start(X[:, :, P:512], x_b[:, :, P:512])
                for nf in range(1, NF):
                    s = slice(nf * 512, (nf + 1) * 512)
                    nc.sync.dma_start(X[:, :, s], x_b[:, :, s])

            def emit_cast(b):
                """f32->fp8 natural-layout copy: matmul-2's moving operand
                and the fp8-transpose source. Emitted at the start of
                batch b's own cycle (single X8 buffer free right then).
                Chunk order starts at bf_cut's chunk so the first fp8
                transposes of phase A unblock earliest."""
                X = Xs[b]
                X8 = x8pool.tile([P, CO, N], FP8, tag="X8")
                X8s[b] = X8
                first_chunk = o["bf_cut"] // 4
                order = [(first_chunk + i) % NF for i in range(NF)]
                for nf in order:
                    s = slice(nf * 512, (nf + 1) * 512)
                    for co in range(CO):
                        eng_copy(o["cast_engine"], X8[:, co, s], X[:, co, s])

            def emit_trans(b, kc, evac=None):
                """Transpose the kc-th 128-col slab of x_b into half of an
                fp8 xt pair tile. kc >= bf_cut reads fp8 X8; below reads
                f32 X (no cast dependency -- used by the cross-batch
                filler and the first A-phase groups)."""
                src8 = kc >= o["bf_cut"]
                ks = slice(kc * P, (kc + 1) * P)
                t = kc // 2
                if (b, t) not in xts:
                    xts[(b, t)] = xtp.tile([P, 2, C], FP8, tag="xt",
                                           name="xt_pair")
                xt_pair = xts[(b, t)]
                src = X8s[b] if src8 else Xs[b]
                idt = ident8 if src8 else ident
                # fp8 transposes must write with element step 2 (HW rule),
                # so the PSUM tile is viewed as [P, 2, C] with the value in
                # slot 0 of each 2-byte granule; the evac read de-interleaves.
                # Tiles padded to a full 2KB PSUM bank so the psx ring
                # buffers never share a bank (PE-W vs evac-R collide at
                # bank granularity)
                if src8:
                    ps_x = psum_xt.tile(
                        [P, 2 * C], FP8, tag="psx", name="ps_x",
                        padded_shape=[P, 2048],
                    )
                    v = ps_x.rearrange("p (c two) -> p two c", two=2)
                    for co in range(CO):
                        nc.tensor.transpose(
                            v[:, 0, co * P:(co + 1) * P], src[:, co, ks], idt
                        )
                    ps_rd = v[:, 0, :]
                else:
                    ps_x = psum_xt.tile([P, C], F32, tag="psx", name="ps_x")
                    for co in range(CO):
                        nc.tensor.transpose(
                            ps_x[:, co * P:(co + 1) * P], src[:, co, ks], idt
                        )
                    ps_rd = ps_x[:, :]
                eng_copy(evac or o["evac_engine"], xt_pair[:, kc % 2, :], ps_rd)

            def emit_mm1(b, t, E):
                xt_pair = xts.pop((b, t))
                for ic in range(CO):
                    nc.tensor.matmul(
                        E[:, ic, ic * P:],
                        xt_pair[:, :, ic * P:(ic + 1) * P],
                        xt_pair[:, :, ic * P:],
                        start=(t == 0),
                        stop=(t == KP - 1),
                        perf_mode=DR,
                    )

            def emit_mirror(b, E):
                for jc in range(1, CO):
                    for ic in range(jc):
                        stg = xtp.tile([P, P], F32, tag="mirror_stage",
                                       bufs=1)
                        nc.scalar.copy(
                            stg[:, :], E[:, ic, jc * P:(jc + 1) * P]
                        )
                        nc.tensor.matmul(
                            E[:, jc, ic * P:(ic + 1) * P],
                            stg[:, :],
                            ident,
                            is_transpose=True,
                            skip_group_check=True,
                        )

            def emit_softmax(b, E):
                """Two-pass: pass 1 computes Z[i] = sum_j exp(mn_i - E_ij)
                (output discarded), pass 2 writes tS = exp(mn - E +
                ln(gamma/Z)) = (gamma/Z) * exp(mn - E) -- folding the
                softmax normalization AND the gamma scale into the exp
                bias, so the mm2 evacuation is a plain residual add.
                gamma=0 -> ln(0) = -inf -> tS identically 0 -> y = x
                exactly. (Requires gamma >= 0.)"""
                mn = stats.tile([P, CO], F32, tag="mn")
                zs = stats.tile([P, CO], F32, tag="zs")
                lng = stats.tile([P, 1], F32, tag="lng")
                b2 = stats.tile([P, CO], F32, tag="b2")
                tS = tpool.tile([P, CO, C], TSDT, tag="t")
                nc.scalar.activation(lng[:, :], g_col[:, :1], AF.Ln)
                for ic in range(CO):
                    nc.vector.tensor_reduce(
                        mn[:, ic:ic + 1], E[:, ic, :], AX.X, ALU.min
                    )
                for ic in range(CO):
                    nc.scalar.activation(
                        tS[:, ic, :], E[:, ic, :], AF.Exp,
                        bias=mn[:, ic:ic + 1], scale=-1.0,
                        accum_out=zs[:, ic:ic + 1],
                    )
                lnz = stats.tile([P, CO], F32, tag="lnz")
                nc.scalar.activation(lnz[:, :], zs[:, :], AF.Ln)
                nc.vector.tensor_sub(b2[:, :], mn[:, :], lnz[:, :])
                nc.vector.tensor_scalar_add(b2[:, :], b2[:, :], lng[:, :1])
                for ic in range(CO):
                    nc.scalar.activation(
                        tS[:, ic, :], E[:, ic, :], AF.Exp,
                        bias=b2[:, ic:ic + 1], scale=-1.0,
                    )
                return tS

            def emit_tT(b, tS):
                tT = ttpool.tile([P, CO, C], FP8, tag="tT")
                idt = ident16 if o["ts_bf16"] else ident
                for jc in range(CO):
                    ps_t = psum_acc.tile(
                        [P, C], TSDT, tag="acc", name="ps_t",
                        padded_shape=[P, 1024] if o["ts_bf16"] else None,
                    )
                    for ic in range(CO):
                        nc.tensor.transpose(
                            ps_t[:, ic * P:(ic + 1) * P],
                            tS[:, ic, jc * P:(jc + 1) * P],
                            idt,
                        )
                    eng_copy(o["tt_evac"], tT[:, jc, :], ps_t[:, :])
                return tT

            def emit_mm2(b, tT):
                X, X8 = Xs[b], X8s[b]
                y_b = y_out[b].rearrange("(co p) n -> p co n", p=P)
                # E's PSUM region is dead during mm2 (softmax already read
                # it); borrow two of its banks so ps2 rotates over mm2_rot
                # banks, keeping the PE ahead of the evac latency.
                ps2e = psum_e.tile([P, 2, C], F32, tag="E", name="ps2e")
                rot = o["mm2_rot"]
                for ic in range(CO):
                    for nf in range(NF):
                        ns = slice(nf * 512, (nf + 1) * 512)
                        g = ic * NF + nf
                        r = g % rot
                        if r >= 2:
                            ps2 = ps2e[:, r - 2, :]
                        else:
                            ps2 = psum_acc.tile([P, C], F32, tag="acc")
                        for t in range(2):
                            nc.tensor.matmul(
                                ps2[:, :512],
                                tT[:, 2 * t:2 * t + 2, ic * P:(ic + 1) * P],
                                X8[:, 2 * t:2 * t + 2, ns],
                                start=(t == 0),
                                stop=(t == 1),
                                perf_mode=DR,
                            )
                        ot = opool.tile([P, 512], F32, tag="o")
                        # tS already carries gamma/Z, so the evacuation is
                        # a single psum+residual add (f32 exact for gamma=0)
                        if o["res_engine"] == "vector":
                            nc.vector.tensor_add(
                                ot[:, :], ps2[:, :512], X[:, ic, ns]
                            )
                        else:
                            nc.gpsimd.tensor_add(
                                ot[:, :], ps2[:, :512], X[:, ic, ns]
                            )
                        nc.sync.dma_start(y_b[:, ic, ns], ot[:, :])
                del Xs[b], X8s[b]

            loop_ctx = (
                tc.For_i(0, reps, 1) if reps > 1 else contextlib.nullcontext()
            )
            with loop_ctx:
                emit_dma(0)
                for b in range(Bs):
                    first = (b == 0)
                    emit_cast(b)
                    E = psum_e.tile([P, CO, C], F32, tag="E")
                    if first:
                        # no prefetched transposes: run 2 ahead of mm1
                        emit_trans(b, 0)
                        emit_trans(b, 1)
                        for kc in range(KC):
                            if kc + 2 < KC:
                                emit_trans(b, kc + 2)
                            if kc == 16 and b + 1 < Bs:
                                emit_dma(b + 1)
                            if kc % 2 == 1:
                                emit_mm1(b, kc // 2, E)
                    else:
                        # kc < PRE were transposed during softmax(b-1)
                        for kc in range(KC):
                            if kc == 0 and b + 1 < Bs:
                                emit_dma(b + 1)
                            if kc + PRE < KC:
                                emit_trans(b, kc + PRE)
                            if kc % 2 == 1:
                                emit_mm1(b, kc // 2, E)
                    emit_mirror(b, E)
                    tS = emit_softmax(b, E)
                    if b + 1 < Bs:
                        for kc in range(PRE):
                            emit_trans(b + 1, kc)
                    tT = emit_tT(b, tS)
                    emit_mm2(b, tT)

    nc.compile()
    return nc


def get_nc(Bs=4, C=512, N=4096, use_f32r=False, reps=1, **opts):
    key = (Bs, C, N, use_f32r, reps, tuple(sorted(opts.items())))
    if key not in _CACHE:
        _CACHE[key] = _build(Bs, C, N, use_f32r, reps, **opts)
    return _CACHE[key]


def kernel(x, gamma):
    """Full inputs in, full output out. x [32, 512, 4096] f32, gamma [1] f32."""
    from concourse.bass_utils import run_bass_kernel_spmd

    x = np.ascontiguousarray(np.asarray(x, dtype=np.float32))
    gamma = np.ascontiguousarray(np.asarray(gamma, dtype=np.float32))
    B, C, N = x.shape
    n_cores = 8
    assert B % n_cores == 0
    Bs = B // n_cores

    nc = get_nc(Bs, C, N)
    in_maps = [
        {"x": x[i * Bs:(i + 1) * Bs], "gamma": gamma} for i in range(n_cores)
    ]
    res = run_bass_kernel_spmd(nc, in_maps, core_ids=list(range(n_cores)))
    return np.concatenate([r["y"] for r in res.results], axis=0)


# revision 18
# speedup vs baseline: 1.2850x; 1.1467x over previous
"""CAM (channel attention) module kernel for Trainium2 (Bass/Tile).

Reference computation (per batch b):
    energy  = x_b @ x_b.T                      # [C, C], contraction over N
    att     = softmax(rowmax(energy) - energy) # row-wise over last axis
    out     = att @ x_b                        # [C, N]
    y_b     = gamma * out + x_b
Identity used: softmax(rowmax(E) - E)[i,j] = exp(mn[i] - E[i,j]) / Z[i]
with mn[i] = min_j E[i,j] (shift invariance of softmax; exact).

Sharding: data-parallel over B across 8 NeuronCores (B=32 -> 4 per core),
gamma replicated, full CxC attention per core.

fp8 design (v2): both matmuls run fp8e4 with perf_mode=DoubleRow (two
K=128 subtiles per matmul, 2 MACs/cell/cycle):
    - xt (x transposed) is built as [P, 2, C] fp8 k-chunk PAIRS: PE
      transposes write PSUM (fp8 from X8 for kc>=bf_cut, f32 from X
      below it -- the f32 path has no cast dependency so next-batch
      prefetch never waits on DVE), evac converts to fp8 pair halves.
    - mm1: E += xt_pair[:,e,:].T @ xt_pair[:,e,:] summed over e, upper
      triangle only; mirrored via PE transpose.
    - softmax on f32 E in PSUM: DVE row-min, ScalarE exp (bf16 tS, Z
      fused), rg = gamma/Z.
    - tT: PE-transpose tS -> bf16 PSUM, ScalarE evac converts to fp8.
    - mm2: per 512-col block, 2 DoubleRow matmuls over jc-pairs with
      moving operand X8 [P, 2, 512]; evac: ScalarE scales by rg,
      residual add of f32 X on res_engine; DMA out.
The residual path stays f32 end to end: rg*psum + x with rg = gamma/Z,
so gamma = 0 reproduces x exactly (all intermediate values finite).

Cross-batch software pipeline (PE program order per steady-state batch):
    [trans(b) kc>=PRE interleaved with mm1(b) pairs] -> mirror(b) ->
    [trans(b+1) kc<PRE : fills the softmax(b) latency] -> tT(b) -> mm2(b)
"""

import contextlib

import numpy as np

P = 128

_CACHE = {}


DEFAULT_OPTS = dict(
    pre=12,        # k-chunks of next batch's transposes emitted early (even)
    xt_bufs=9,     # xT pair SBUF tiles
    o_bufs=12,     # output staging tiles
    x8_bufs=2,     # X8 buffers (2 decouples cast(b) from mm2(b-1) reads)
    cast_scalar=8, # X8 cast chunks (of 32) on ScalarE; rest on DVE
    evac_engine="scalar",   # engine for ps_x -> xt evacuation
    tt_evac="scalar",       # engine for ps_t -> tT evacuation
    mm2_rot=4,     # mm2 PSUM bank rotation depth (2 acc + borrowed E banks)
    bf_cut=12,     # kc >= bf_cut transpose X8 (fp8); below: X (f32).
                   # Must be >= pre (prefetched transposes have no X8 yet).
    ts_bf16=True,  # tS (exp output) in bf16
    timing_io=False,
)


def _build(Bs, C, N, use_f32r=False, reps=1, **opts):
    import concourse.bass as bass  # noqa: F401
    import concourse.tile as tile
    import concourse.mybir as mybir
    from concourse import bacc
    from concourse.masks import make_identity

    o = dict(DEFAULT_OPTS)
    o.update(opts)

    F32 = mybir.dt.float32
    BF16 = mybir.dt.bfloat16
    FP8 = mybir.dt.float8e4
    AF = mybir.ActivationFunctionType
    ALU = mybir.AluOpType
    AX = mybir.AxisListType
    DR = mybir.MatmulPerfMode.DoubleRow

    assert C == 4 * P and N % 512 == 0
    CO = C // P          # i/j chunks of 128
    KC = N // P          # n chunks of 128 (contraction for energy)
    KP = KC // 2         # k-chunk pairs for DoubleRow
    NF = N // 512        # n chunks of 512 (DMA / matmul-2 free dim)
    PRE = o["pre"]
    assert KC % 2 == 0 and PRE % 2 == 0
    assert o["bf_cut"] >= PRE, "prefetched transposes need the f32 path"
    TSDT = BF16 if o["ts_bf16"] else F32

    nc = bacc.Bacc(None, target_bir_lowering=False, debug=False)
    x_in = nc.dram_tensor("x", [Bs, C, N], F32, kind="ExternalInput")
    g_in = nc.dram_tensor("gamma", [1], F32, kind="ExternalInput")
    y_out = nc.dram_tensor("y", [Bs, C, N], F32, kind="ExternalOutput")

    with tile.TileContext(nc) as tc:
        with (
            tc.tile_pool(name="consts", bufs=1) as consts,
            tc.tile_pool(name="xpool", bufs=2) as xpool,
            tc.tile_pool(name="x8pool", bufs=o["x8_bufs"]) as x8pool,
            tc.tile_pool(name="xtp", bufs=o["xt_bufs"]) as xtp,
            tc.tile_pool(name="tpool", bufs=1) as tpool,
            tc.tile_pool(name="ttpool", bufs=1) as ttpool,
            tc.tile_pool(name="opool", bufs=o["o_bufs"]) as opool,
            tc.tile_pool(name="stats", bufs=1) as stats,
            tc.tile_pool(name="pe", bufs=1, space="PSUM") as psum_e,
            tc.tile_pool(name="pxt", bufs=2, space="PSUM") as psum_xt,
            tc.tile_pool(name="pacc", bufs=2, space="PSUM") as psum_acc,
        ):
            ident = consts.tile([P, P], F32)
            make_identity(nc, ident)
            ident16 = consts.tile([P, P], BF16)
            nc.vector.tensor_copy(ident16[:, :], ident[:, :])
            ident8 = consts.tile([P, P], FP8)
            nc.vector.tensor_copy(ident8[:, :], ident[:, :])
            g_sb = consts.tile([1, 1], F32)
            nc.sync.dma_start(g_sb[:, :], g_in[:].rearrange("(a b) -> a b", a=1))
            g_col = consts.tile([P, 1], F32)
            nc.gpsimd.partition_broadcast(g_col[:, :], g_sb[:1, :1])

            def eng_copy(engine, out, in_):
                if engine == "vector":
                    nc.vector.tensor_copy(out, in_)
                elif engine == "scalar":
                    nc.scalar.copy(out, in_)
                else:
                    nc.gpsimd.tensor_copy(out, in_)

            # per-batch live tiles
            Xs, X8s, xts = {}, {}, {}

            def emit_dma(b):
                """DMA x_b in 512-col chunks."""
                x_b = x_in[b].rearrange("(co p) n -> p co n", p=P)
                X = xpool.tile([P, CO, N], F32, tag="X")
                Xs[b] = X
                # first 128 columns land alone so transposes start early
                nc.sync.dma_start(X[:, :, 0:P], x_b[:, :, 0:P])
                nc.sync.dma_start(X[:, :, P:512], x_b[:, :, P:512])
                for nf in range(1, NF):
                    s = slice(nf * 512, (nf + 1) * 512)
                    nc.sync.dma_start(X[:, :, s], x_b[:, :, s])

            def emit_cast(b):
                """f32->fp8 natural-layout copy: matmul-2's moving operand
                and the fp8-transpose source. Emitted at the start of
                batch b's own cycle (single X8 buffer free right then).
                Chunk order starts at bf_cut's chunk so the first fp8
                transposes of phase A unblock earliest."""
                X = Xs[b]
                X8 = x8pool.tile([P, CO, N], FP8, tag="X8")
                X8s[b] = X8
                first_chunk = o["bf_cut"] // 4
                order = [(first_chunk + i) % NF for i in range(NF)]
                idx = 0
                for nf in order:
                    s = slice(nf * 512, (nf + 1) * 512)
                    for co in range(CO):
                        # the first few (soonest-needed) chunks go on
                        # ScalarE, which is idle during mm2(b-1); the rest
                        # on DVE, which is idle during phase A
                        eng = "scalar" if idx < o["cast_scalar"] else "vector"
                        eng_copy(eng, X8[:, co, s], X[:, co, s])
                        idx += 1

            def emit_trans(b, kc, evac=None):
                """Transpose the kc-th 128-col slab of x_b into half of an
                fp8 xt pair tile. kc >= bf_cut reads fp8 X8; below reads
                f32 X (no cast dependency -- used by the cross-batch
                filler and the first A-phase groups)."""
                src8 = kc >= o["bf_cut"]
                ks = slice(kc * P, (kc + 1) * P)
                t = kc // 2
                if (b, t) not in xts:
                    xts[(b, t)] = xtp.tile([P, 2, C], FP8, tag="xt",
                                           name="xt_pair")
                xt_pair = xts[(b, t)]
                src = X8s[b] if src8 else Xs[b]
                idt = ident8 if src8 else ident
                # fp8 transposes must write with element step 2 (HW rule),
                # so the PSUM tile is viewed as [P, 2, C] with the value in
                # slot 0 of each 2-byte granule; the evac read de-interleaves.
                # Tiles padded to a full 2KB PSUM bank so the psx ring
                # buffers never share a bank (PE-W vs evac-R collide at
                # bank granularity)
                if src8:
                    ps_x = psum_xt.tile(
                        [P, 2 * C], FP8, tag="psx", name="ps_x",
                        padded_shape=[P, 2048],
                    )
                    v = ps_x.rearrange("p (c two) -> p two c", two=2)
                    for co in range(CO):
                        nc.tensor.transpose(
                            v[:, 0, co * P:(co + 1) * P], src[:, co, ks], idt
                        )
                    ps_rd = v[:, 0, :]
                else:
                    ps_x = psum_xt.tile([P, C], F32, tag="psx", name="ps_x")
                    for co in range(CO):
                        nc.tensor.transpose(
                            ps_x[:, co * P:(co + 1) * P], src[:, co, ks], idt
                        )
                    ps_rd = ps_x[:, :]
                eng_copy(evac or o["evac_engine"], xt_pair[:, kc % 2, :], ps_rd)

            def emit_mm1(b, t, E):
                xt_pair = xts.pop((b, t))
                for ic in range(CO):
                    nc.tensor.matmul(
                        E[:, ic, ic * P:],
                        xt_pair[:, :, ic * P:(ic + 1) * P],
                        xt_pair[:, :, ic * P:],
                        start=(t == 0),
                        stop=(t == KP - 1),
                        perf_mode=DR,
                    )

            def emit_mirror(b, E):
                for jc in range(1, CO):
                    for ic in range(jc):
                        stg = xtp.tile([P, P], F32, tag="mirror_stage",
                                       bufs=1)
                        nc.scalar.copy(
                            stg[:, :], E[:, ic, jc * P:(jc + 1) * P]
                        )
                        nc.tensor.matmul(
                            E[:, jc, ic * P:(ic + 1) * P],
                            stg[:, :],
                            ident,
                            is_transpose=True,
                            skip_group_check=True,
                        )

            def emit_softmax(b, E):
                """mn row-min (DVE), one exp pass (tS = exp(mn-E), Z fused),
                rg = gamma/Z, then D[:, ic, :] = diag(rg[:, ic]) bf16. The
                gamma/Z scale is applied FREE inside the tT matmuls (tS.T @
                diag(rg) instead of tS.T @ I). gamma=0 -> D = 0 -> tT = 0
                -> y = x exactly."""
                mn = stats.tile([P, CO], F32, tag="mn")
                zs = stats.tile([P, CO], F32, tag="zs")
                rg = stats.tile([P, CO], F32, tag="rg")
                tS = tpool.tile([P, CO, C], TSDT, tag="t")
                for ic in range(CO):
                    nc.vector.tensor_reduce(
                        mn[:, ic:ic + 1], E[:, ic, :], AX.X, ALU.min
                    )
                for ic in range(CO):
                    nc.scalar.activation(
                        tS[:, ic, :], E[:, ic, :], AF.Exp,
                        bias=mn[:, ic:ic + 1], scale=-1.0,
                        accum_out=zs[:, ic:ic + 1],
                    )
                nc.vector.reciprocal(rg[:, :], zs[:, :])
                nc.vector.tensor_scalar_mul(rg[:, :], rg[:, :], g_col[:, :1])
                D = stats.tile([P, CO, P], BF16, tag="diag")
                for ic in range(CO):
                    nc.vector.tensor_scalar_mul(
                        D[:, ic, :], ident16[:, :], rg[:, ic:ic + 1]
                    )
                return tS, D

            def emit_tT(b, tS, D):
                tT = ttpool.tile([P, CO, C], FP8, tag="tT")
                for jc in range(CO):
                    ps_t = psum_acc.tile([P, C], F32, tag="acc", name="ps_t")
                    for ic in range(CO):
                        # ps_t[j, i] = sum_p tS[p, j] * D[p, i]
                        #            = tS[i, j] * rg[i]  (D diagonal)
                        nc.tensor.matmul(
                            ps_t[:, ic * P:(ic + 1) * P],
                            tS[:, ic, jc * P:(jc + 1) * P],
                            D[:, ic, :],
                        )
                    eng_copy(o["tt_evac"], tT[:, jc, :], ps_t[:, :])
                return tT

            def emit_mm2(b, tT):
                X, X8 = Xs[b], X8s[b]
                y_b = y_out[b].rearrange("(co p) n -> p co n", p=P)
                # E's PSUM region is dead during mm2 (softmax already read
                # it); borrow two of its banks so ps2 rotates over mm2_rot
                # banks, keeping the PE ahead of the evac latency.
                ps2e = psum_e.tile([P, 2, C], F32, tag="E", name="ps2e")
                rot = o["mm2_rot"]
                for ic in range(CO):
                    for nf in range(NF):
                        ns = slice(nf * 512, (nf + 1) * 512)
                        g = ic * NF + nf
                        r = g % rot
                        if r >= 2:
                            ps2 = ps2e[:, r - 2, :]
                        else:
                            ps2 = psum_acc.tile([P, C], F32, tag="acc")
                        for t in range(2):
                            nc.tensor.matmul(
                                ps2[:, :512],
                                tT[:, 2 * t:2 * t + 2, ic * P:(ic + 1) * P],
                                X8[:, 2 * t:2 * t + 2, ns],
                                start=(t == 0),
                                stop=(t == 1),
                                perf_mode=DR,
                            )
                        ot = opool.tile([P, 512], F32, tag="o")
                        # tT already carries gamma/Z, so the evacuation is
                        # one psum+residual add (f32, exact for gamma=0)
                        nc.vector.tensor_add(
                            ot[:, :], ps2[:, :512], X[:, ic, ns]
                        )
                        nc.sync.dma_start(y_b[:, ic, ns], ot[:, :])
                del Xs[b], X8s[b]

            loop_ctx = (
                tc.For_i(0, reps, 1) if reps > 1 else contextlib.nullcontext()
            )
            with loop_ctx:
                emit_dma(0)
                for b in range(Bs):
                    first = (b == 0)
                    emit_cast(b)
                    E = psum_e.tile([P, CO, C], F32, tag="E")
                    if first:
                        # no prefetched transposes: run 2 ahead of mm1
                        emit_trans(b, 0)
                        emit_trans(b, 1)
                        for kc in range(KC):
                            if kc + 2 < KC:
                                emit_trans(b, kc + 2)
                            if kc == 16 and b + 1 < Bs:
                                emit_dma(b + 1)
                            if kc % 2 == 1:
                                emit_mm1(b, kc // 2, E)
                    else:
                        # kc < PRE were transposed during softmax(b-1)
                        for kc in range(KC):
                            if kc == 0 and b + 1 < Bs:
                                emit_dma(b + 1)
                            if kc + PRE < KC:
                                emit_trans(b, kc + PRE)
                            if kc % 2 == 1:
                                emit_mm1(b, kc // 2, E)
                    emit_mirror(b, E)
                    tS, D = emit_softmax(b, E)
                    if b + 1 < Bs:
                        for kc in range(PRE):
                            emit_trans(b + 1, kc)
                    tT = emit_tT(b, tS, D)
                    emit_mm2(b, tT)

    nc.compile()
    return nc


def get_nc(Bs=4, C=512, N=4096, use_f32r=False, reps=1, **opts):
    key = (Bs, C, N, use_f32r, reps, tuple(sorted(opts.items())))
    if key not in _CACHE:
        _CACHE[key] = _build(Bs, C, N, use_f32r, reps, **opts)
    return _CACHE[key]


def kernel(x, gamma):
    """Full inputs in, full output out. x [32, 512, 4096] f32, gamma [1] f32."""
    from concourse.bass_utils import run_bass_kernel_spmd

    x = np.ascontiguousarray(np.asarray(x, dtype=np.float32))
    gamma = np.ascontiguousarray(np.asarray(gamma, dtype=np.float32))
    B, C, N = x.shape
    n_cores = 8
    assert B % n_cores == 0
    Bs = B // n_cores

    nc = get_nc(Bs, C, N)
    in_maps = [
        {"x": x[i * Bs:(i + 1) * Bs], "gamma": gamma} for i in range(n_cores)
    ]
    res = run_bass_kernel_spmd(nc, in_maps, core_ids=list(range(n_cores)))
    return np.concatenate([r["y"] for r in res.results], axis=0)


# revision 27
# speedup vs baseline: 1.3174x; 1.0252x over previous
"""CAM (channel attention) module kernel for Trainium2 (Bass/Tile).

Reference computation (per batch b):
    energy  = x_b @ x_b.T                      # [C, C], contraction over N
    att     = softmax(rowmax(energy) - energy) # row-wise over last axis
    out     = att @ x_b                        # [C, N]
    y_b     = gamma * out + x_b
Identity used: softmax(rowmax(E) - E)[i,j] = exp(mn[i] - E[i,j]) / Z[i]
with mn[i] = min_j E[i,j] (shift invariance of softmax; exact).

Sharding: data-parallel over B across 8 NeuronCores (B=32 -> 4 per core),
gamma replicated, full CxC attention per core.

fp8 design (v2): both matmuls run fp8e4 with perf_mode=DoubleRow (two
K=128 subtiles per matmul, 2 MACs/cell/cycle):
    - xt (x transposed) is built as [P, 2, C] fp8 k-chunk PAIRS: PE
      transposes write PSUM (fp8 from X8 for kc>=bf_cut, f32 from X
      below it -- the f32 path has no cast dependency so next-batch
      prefetch never waits on DVE), evac converts to fp8 pair halves.
    - mm1: E += xt_pair[:,e,:].T @ xt_pair[:,e,:] summed over e, upper
      triangle only; mirrored via PE transpose.
    - softmax on f32 E in PSUM: DVE row-min, ScalarE exp (bf16 tS, Z
      fused), rg = gamma/Z.
    - tT: PE-transpose tS -> bf16 PSUM, ScalarE evac converts to fp8.
    - mm2: per 512-col block, 2 DoubleRow matmuls over jc-pairs with
      moving operand X8 [P, 2, 512]; evac: ScalarE scales by rg,
      residual add of f32 X on res_engine; DMA out.
The residual path stays f32 end to end: rg*psum + x with rg = gamma/Z,
so gamma = 0 reproduces x exactly (all intermediate values finite).

Cross-batch software pipeline (PE program order per steady-state batch):
    [trans(b) kc>=PRE interleaved with mm1(b) pairs] -> mirror(b) ->
    [trans(b+1) kc<PRE : fills the softmax(b) latency] -> tT(b) -> mm2(b)
"""

import contextlib

import numpy as np

P = 128

_CACHE = {}


DEFAULT_OPTS = dict(
    pre=12,        # k-chunks of next batch's transposes emitted early (even)
    xt_bufs=9,     # xT pair SBUF tiles
    o_bufs=12,     # output staging tiles
    x8_bufs=2,     # X8 buffers (2 decouples cast(b) from mm2(b-1) reads)
    cast_scalar=8, # X8 cast chunks (of 32) on ScalarE; rest on DVE
    evac_engine="scalar",   # engine for ps_x -> xt evacuation
    tt_evac="scalar",       # engine for ps_t -> tT evacuation
    mm2_rot=4,     # mm2 PSUM bank rotation depth (2 acc + borrowed E banks)
    pe_res=True,   # odd mm2 groups: PE adds residual, ScalarE evacs
    bf_cut=12,     # kc >= bf_cut transpose X8 (fp8); below: X (f32).
                   # Must be >= pre (prefetched transposes have no X8 yet).
    ts_bf16=True,  # tS (exp output) in bf16
    timing_io=False,
)


def _build(Bs, C, N, use_f32r=False, reps=1, **opts):
    import concourse.bass as bass  # noqa: F401
    import concourse.tile as tile
    import concourse.mybir as mybir
    from concourse import bacc
    from concourse.masks import make_identity

    o = dict(DEFAULT_OPTS)
    o.update(opts)

    F32 = mybir.dt.float32
    F32R = mybir.dt.float32r
    BF16 = mybir.dt.bfloat16
    FP8 = mybir.dt.float8e4
    AF = mybir.ActivationFunctionType
    ALU = mybir.AluOpType
    AX = mybir.AxisListType
    DR = mybir.MatmulPerfMode.DoubleRow

    assert C == 4 * P and N % 512 == 0
    CO = C // P          # i/j chunks of 128
    KC = N // P          # n chunks of 128 (contraction for energy)
    KP = KC // 2         # k-chunk pairs for DoubleRow
    NF = N // 512        # n chunks of 512 (DMA / matmul-2 free dim)
    PRE = o["pre"]
    assert KC % 2 == 0 and PRE % 2 == 0
    assert o["bf_cut"] >= PRE, "prefetched transposes need the f32 path"
    TSDT = BF16 if o["ts_bf16"] else F32

    nc = bacc.Bacc(None, target_bir_lowering=False, debug=False)
    x_in = nc.dram_tensor("x", [Bs, C, N], F32, kind="ExternalInput")
    g_in = nc.dram_tensor("gamma", [1], F32, kind="ExternalInput")
    y_out = nc.dram_tensor("y", [Bs, C, N], F32, kind="ExternalOutput")

    with tile.TileContext(nc) as tc:
        with (
            tc.tile_pool(name="consts", bufs=1) as consts,
            tc.tile_pool(name="xpool", bufs=2) as xpool,
            tc.tile_pool(name="x8pool", bufs=o["x8_bufs"]) as x8pool,
            tc.tile_pool(name="xtp", bufs=o["xt_bufs"]) as xtp,
            tc.tile_pool(name="tpool", bufs=1) as tpool,
            tc.tile_pool(name="ttpool", bufs=1) as ttpool,
            tc.tile_pool(name="opool", bufs=o["o_bufs"]) as opool,
            tc.tile_pool(name="stats", bufs=1) as stats,
            tc.tile_pool(name="pe", bufs=1, space="PSUM") as psum_e,
            tc.tile_pool(name="pxt", bufs=2, space="PSUM") as psum_xt,
            tc.tile_pool(name="pacc", bufs=2, space="PSUM") as psum_acc,
        ):
            ident = consts.tile([P, P], F32)
            make_identity(nc, ident)
            ident16 = consts.tile([P, P], BF16)
            nc.vector.tensor_copy(ident16[:, :], ident[:, :])
            ident8 = consts.tile([P, P], FP8)
            nc.vector.tensor_copy(ident8[:, :], ident[:, :])
            identr = consts.tile([P, P], F32R)
            nc.vector.tensor_copy(identr[:, :], ident[:, :])
            g_sb = consts.tile([1, 1], F32)
            nc.sync.dma_start(g_sb[:, :], g_in[:].rearrange("(a b) -> a b", a=1))
            g_col = consts.tile([P, 1], F32)
            nc.gpsimd.partition_broadcast(g_col[:, :], g_sb[:1, :1])

            def eng_copy(engine, out, in_):
                if engine == "vector":
                    nc.vector.tensor_copy(out, in_)
                elif engine == "scalar":
                    nc.scalar.copy(out, in_)
                else:
                    nc.gpsimd.tensor_copy(out, in_)

            # per-batch live tiles
            Xs, X8s, xts = {}, {}, {}

            def emit_dma(b):
                """DMA x_b in 512-col chunks. The tile is declared f32r
                (same bytes as f32) so the pe_res matmul can consume it
                natively; f32 users read it through .bitcast(F32)."""
                x_b = (
                    x_in[b].bitcast(F32R).rearrange("(co p) n -> p co n", p=P)
                )
                X = xpool.tile([P, CO, N], F32R, tag="X")
                Xs[b] = X
                # first 128 columns land alone so transposes start early
                nc.sync.dma_start(X[:, :, 0:P], x_b[:, :, 0:P])
                nc.sync.dma_start(X[:, :, P:512], x_b[:, :, P:512])
                for nf in range(1, NF):
                    s = slice(nf * 512, (nf + 1) * 512)
                    nc.sync.dma_start(X[:, :, s], x_b[:, :, s])

            def emit_cast(b):
                """f32->fp8 natural-layout copy: matmul-2's moving operand
                and the fp8-transpose source. Emitted at the start of
                batch b's own cycle (single X8 buffer free right then).
                Chunk order starts at bf_cut's chunk so the first fp8
                transposes of phase A unblock earliest."""
                X = Xs[b]
                X8 = x8pool.tile([P, CO, N], FP8, tag="X8")
                X8s[b] = X8
                first_chunk = o["bf_cut"] // 4
                order = [(first_chunk + i) % NF for i in range(NF)]
                idx = 0
                for nf in order:
                    s = slice(nf * 512, (nf + 1) * 512)
                    for co in range(CO):
                        # the first few (soonest-needed) chunks go on
                        # ScalarE, which is idle during mm2(b-1); the rest
                        # on DVE, which is idle during phase A
                        eng = "scalar" if idx < o["cast_scalar"] else "vector"
                        eng_copy(eng, X8[:, co, s], X.bitcast(F32)[:, co, s])
                        idx += 1

            def emit_trans(b, kc, evac=None):
                """Transpose the kc-th 128-col slab of x_b into half of an
                fp8 xt pair tile. kc >= bf_cut reads fp8 X8; below reads
                f32 X (no cast dependency -- used by the cross-batch
                filler and the first A-phase groups)."""
                src8 = kc >= o["bf_cut"]
                ks = slice(kc * P, (kc + 1) * P)
                t = kc // 2
                if (b, t) not in xts:
                    xts[(b, t)] = xtp.tile([P, 2, C], FP8, tag="xt",
                                           name="xt_pair")
                xt_pair = xts[(b, t)]
                src = X8s[b] if src8 else Xs[b].bitcast(F32)
                idt = ident8 if src8 else ident
                # fp8 transposes must write with element step 2 (HW rule),
                # so the PSUM tile is viewed as [P, 2, C] with the value in
                # slot 0 of each 2-byte granule; the evac read de-interleaves.
                # Tiles padded to a full 2KB PSUM bank so the psx ring
                # buffers never share a bank (PE-W vs evac-R collide at
                # bank granularity)
                if src8:
                    ps_x = psum_xt.tile(
                        [P, 2 * C], FP8, tag="psx", name="ps_x",
                        padded_shape=[P, 2048],
                    )
                    v = ps_x.rearrange("p (c two) -> p two c", two=2)
                    for co in range(CO):
                        nc.tensor.transpose(
                            v[:, 0, co * P:(co + 1) * P], src[:, co, ks], idt
                        )
                    ps_rd = v[:, 0, :]
                else:
                    ps_x = psum_xt.tile([P, C], F32, tag="psx", name="ps_x")
                    for co in range(CO):
                        nc.tensor.transpose(
                            ps_x[:, co * P:(co + 1) * P], src[:, co, ks], idt
                        )
                    ps_rd = ps_x[:, :]
                eng_copy(evac or o["evac_engine"], xt_pair[:, kc % 2, :], ps_rd)

            def emit_mm1(b, t, E):
                xt_pair = xts.pop((b, t))
                for ic in range(CO):
                    nc.tensor.matmul(
                        E[:, ic, ic * P:],
                        xt_pair[:, :, ic * P:(ic + 1) * P],
                        xt_pair[:, :, ic * P:],
                        start=(t == 0),
                        stop=(t == KP - 1),
                        perf_mode=DR,
                    )

            def emit_mirror(b, E):
                for jc in range(1, CO):
                    for ic in range(jc):
                        stg = xtp.tile([P, P], F32, tag="mirror_stage",
                                       bufs=1)
                        nc.scalar.copy(
                            stg[:, :], E[:, ic, jc * P:(jc + 1) * P]
                        )
                        nc.tensor.matmul(
                            E[:, jc, ic * P:(ic + 1) * P],
                            stg[:, :],
                            ident,
                            is_transpose=True,
                            skip_group_check=True,
                        )

            def emit_softmax(b, E):
                """mn row-min (DVE), one exp pass (tS = exp(mn-E), Z fused),
                rg = gamma/Z, then D[:, ic, :] = diag(rg[:, ic]) bf16. The
                gamma/Z scale is applied FREE inside the tT matmuls (tS.T @
                diag(rg) instead of tS.T @ I). gamma=0 -> D = 0 -> tT = 0
                -> y = x exactly."""
                mn = stats.tile([P, CO], F32, tag="mn")
                zs = stats.tile([P, CO], F32, tag="zs")
                rg = stats.tile([P, CO], F32, tag="rg")
                tS = tpool.tile([P, CO, C], TSDT, tag="t")
                for ic in range(CO):
                    nc.vector.tensor_reduce(
                        mn[:, ic:ic + 1], E[:, ic, :], AX.X, ALU.min
                    )
                D = stats.tile([P, CO, P], BF16, tag="diag")
                for ic in range(CO):
                    nc.scalar.activation(
                        tS[:, ic, :], E[:, ic, :], AF.Exp,
                        bias=mn[:, ic:ic + 1], scale=-1.0,
                        accum_out=zs[:, ic:ic + 1],
                    )
                    # per-ic so D[ic] is ready right after exp(ic), not
                    # after the whole exp sweep (shortens the tT chain)
                    nc.vector.reciprocal(rg[:, ic:ic + 1], zs[:, ic:ic + 1])
                    nc.vector.tensor_scalar_mul(
                        rg[:, ic:ic + 1], rg[:, ic:ic + 1], g_col[:, :1]
                    )
                    nc.vector.tensor_scalar_mul(
                        D[:, ic, :], ident16[:, :], rg[:, ic:ic + 1]
                    )
                return tS, D

            def emit_tT(b, tS, D):
                tT = ttpool.tile([P, CO, C], FP8, tag="tT")
                for jc in range(CO):
                    ps_t = psum_acc.tile([P, C], F32, tag="acc", name="ps_t")
                    for ic in range(CO):
                        # ps_t[j, i] = sum_p tS[p, j] * D[p, i]
                        #            = tS[i, j] * rg[i]  (D diagonal)
                        nc.tensor.matmul(
                            ps_t[:, ic * P:(ic + 1) * P],
                            tS[:, ic, jc * P:(jc + 1) * P],
                            D[:, ic, :],
                        )
                    eng_copy(o["tt_evac"], tT[:, jc, :], ps_t[:, :])
                return tT

            def emit_mm2(b, tT):
                X, X8 = Xs[b], X8s[b]
                y_b = y_out[b].rearrange("(co p) n -> p co n", p=P)
                # E's PSUM region is dead during mm2 (softmax already read
                # it); borrow two of its banks so ps2 rotates over mm2_rot
                # banks, keeping the PE ahead of the evac latency.
                ps2e = psum_e.tile([P, 2, C], F32, tag="E", name="ps2e")
                rot = o["mm2_rot"]
                for ic in range(CO):
                    for nf in range(NF):
                        ns = slice(nf * 512, (nf + 1) * 512)
                        g = ic * NF + nf
                        r = g % rot
                        if r >= 2:
                            ps2 = ps2e[:, r - 2, :]
                        else:
                            ps2 = psum_acc.tile([P, C], F32, tag="acc")
                        # alternate evac styles so DVE's adds don't pace
                        # the whole mm2 phase: odd groups let the PE add
                        # the residual (identity @ f32r view of X -- FP22
                        # precision, ~6e-5 rel) and ScalarE does a plain
                        # copy; even groups do the DVE psum+X add.
                        pe_res = g % 2 == 1 and o["pe_res"]
                        for t in range(2):
                            nc.tensor.matmul(
                                ps2[:, :512],
                                tT[:, 2 * t:2 * t + 2, ic * P:(ic + 1) * P],
                                X8[:, 2 * t:2 * t + 2, ns],
                                start=(t == 0),
                                stop=(t == 1) and not pe_res,
                                perf_mode=DR,
                            )
                        if pe_res:
                            nc.tensor.matmul(
                                ps2[:, :512],
                                identr[:, :],
                                X[:, ic, ns],
                                start=False,
                                stop=True,
                            )
                        ot = opool.tile([P, 512], F32, tag="o")
                        if pe_res:
                            nc.scalar.copy(ot[:, :], ps2[:, :512])
                        else:
                            # tT already carries gamma/Z: one psum+residual
                            # add (f32, exact for gamma=0)
                            nc.vector.tensor_add(
                                ot[:, :], ps2[:, :512], X.bitcast(F32)[:, ic, ns]
                            )
                        nc.sync.dma_start(y_b[:, ic, ns], ot[:, :])
                del Xs[b], X8s[b]

            loop_ctx = (
                tc.For_i(0, reps, 1) if reps > 1 else contextlib.nullcontext()
            )
            with loop_ctx:
                emit_dma(0)
                for b in range(Bs):
                    first = (b == 0)
                    emit_cast(b)
                    E = psum_e.tile([P, CO, C], F32, tag="E")
                    if first:
                        # no prefetched transposes: run 2 ahead of mm1
                        emit_trans(b, 0)
                        emit_trans(b, 1)
                        for kc in range(KC):
                            if kc + 2 < KC:
                                emit_trans(b, kc + 2)
                            if kc == 16 and b + 1 < Bs:
                                emit_dma(b + 1)
                            if kc % 2 == 1:
                                emit_mm1(b, kc // 2, E)
                    else:
                        # kc < PRE were transposed during softmax(b-1)
                        for kc in range(KC):
                            if kc == 0 and b + 1 < Bs:
                                emit_dma(b + 1)
                            if kc + PRE < KC:
                                emit_trans(b, kc + PRE)
                            if kc % 2 == 1:
                                emit_mm1(b, kc // 2, E)
                    emit_mirror(b, E)
                    tS, D = emit_softmax(b, E)
                    if b + 1 < Bs:
                        for kc in range(PRE):
                            emit_trans(b + 1, kc)
                    tT = emit_tT(b, tS, D)
                    emit_mm2(b, tT)

    nc.compile()
    return nc


def get_nc(Bs=4, C=512, N=4096, use_f32r=False, reps=1, **opts):
    key = (Bs, C, N, use_f32r, reps, tuple(sorted(opts.items())))
    if key not in _CACHE:
        _CACHE[key] = _build(Bs, C, N, use_f32r, reps, **opts)
    return _CACHE[key]


def kernel(x, gamma):
    """Full inputs in, full output out. x [32, 512, 4096] f32, gamma [1] f32."""
    from concourse.bass_utils import run_bass_kernel_spmd

    x = np.ascontiguousarray(np.asarray(x, dtype=np.float32))
    gamma = np.ascontiguousarray(np.asarray(gamma, dtype=np.float32))
    B, C, N = x.shape
    n_cores = 8
    assert B % n_cores == 0
    Bs = B // n_cores

    nc = get_nc(Bs, C, N)
    in_maps = [
        {"x": x[i * Bs:(i + 1) * Bs], "gamma": gamma} for i in range(n_cores)
    ]
    res = run_bass_kernel_spmd(nc, in_maps, core_ids=list(range(n_cores)))
    return np.concatenate([r["y"] for r in res.results], axis=0)
